# revision 12
# baseline (speedup 1.0000x reference)
"""Distributed causal multi-head attention for TRN2 (8 NeuronCores).

Problem: x[2,2048,1024] -> MHA(16 heads, dk=dv=64, causal) -> out[2,2048,1024].

Sharding: 2-way data parallel over batch x 4-way tensor parallel over heads.
Core c = 4*b + g handles batch b, heads 4g..4g+3 (columns 256g..256g+256 of
Wq/Wk/Wv, rows 256g..256g+256 of Wo). Each core computes a partial output
projection Y_bg = O_g @ Wo_g; the host sums the 4 partials per batch
(unsharding a sum-sharded tensor) and stacks the batches.

Device kernel (per core, transpose-free):
  - host passes x^T (d-major) in bf16, so QKV projections contract over d
    with no on-device transpose.
  - Q^T,K^T [j,s] layouts feed scores S^T = K^T.T @ Q^T directly; V in
    natural [k,v] layout feeds O^T = [V|1].T @ P^T; the appended ones
    column produces softmax denominators in the same matmul.
  - softmax without max-subtraction: scores are ~N(0,0.17) here, exp is
    safe in fp32; causal masking via k-tile skipping, column-narrowed
    matmuls and one 128x128 tril multiply per diagonal block.
  - out projection Y = O^T.T @ Wo lands in natural [s,m] layout for DMA.
"""

import numpy as np
import ml_dtypes

from concourse import bacc, mybir, tile
from concourse.bass_utils import run_bass_kernel_spmd

BF16 = mybir.dt.bfloat16
F32 = mybir.dt.float32
AF = mybir.ActivationFunctionType
ALU = mybir.AluOpType

B, S, D = 2, 2048, 1024
NH, DK = 16, 64
HPC = 4                      # heads per core
JC = HPC * DK                # 256 local q/k/v columns
N_CORES = 8
SC = 512                     # q-chunk (matmul moving free dim)
NQ = S // SC                 # 4 q-chunks
NKT = S // 128               # 16 k-tiles
NST = S // 128               # 16 s-tiles


def _body(tc, io):
    nc = tc.nc
    with (
        tc.tile_pool(name="persist", bufs=1) as pp,
        tc.tile_pool(name="proj_ps", bufs=2, space="PSUM") as proj_ps,
        tc.tile_pool(name="s_ps", bufs=1, space="PSUM") as s_ps,
        tc.tile_pool(name="o_ps", bufs=2, space="PSUM") as o_ps,
        tc.tile_pool(name="pbuf", bufs=4) as p_pool,
        tc.tile_pool(name="ybuf", bufs=4) as y_pool,
        tc.tile_pool(name="small", bufs=4) as small_pool,
    ):
        # ---- constant / weight loads -------------------------------------
        xT_t = []
        for d in range(8):
            t = pp.tile([128, S], BF16, name=f"xT{d}", tag=f"xT{d}")
            nc.sync.dma_start(t[:], io["xT"][d * 128:(d + 1) * 128, :])
            xT_t.append(t)

        def load_w(key, n_free):
            tiles = []
            for d in range(8):
                t = pp.tile([128, n_free], BF16, name=f"{key}{d}", tag=f"{key}{d}")
                nc.sync.dma_start(t[:], io[key][d * 128:(d + 1) * 128, :])
                tiles.append(t)
            return tiles

        wq_t = load_w("wq", JC)
        wk_t = load_w("wk", JC)
        wv_t = load_w("wv", JC)

        wo_t = []
        for p in range(2):
            t = pp.tile([128, D], BF16, name=f"wo{p}", tag=f"wo{p}")
            nc.sync.dma_start(t[:], io["wo"][p * 128:(p + 1) * 128, :])
            wo_t.append(t)

        bq_sb = pp.tile([128, 2], F32, name="bq_sb", tag="bq_sb")
        nc.sync.dma_start(bq_sb[:], io["bqs"][:, :])
        bk_sb = pp.tile([128, 2], F32, name="bk_sb", tag="bk_sb")
        nc.sync.dma_start(bk_sb[:], io["bks"][:, :])

        bv_row = pp.tile([1, JC], F32, name="bv_row", tag="bv_row")
        nc.sync.dma_start(bv_row[:], io["bvr"][:, :])
        bv_bc = pp.tile([128, JC], F32, name="bv_bc", tag="bv_bc")
        nc.gpsimd.partition_broadcast(bv_bc[:], bv_row[:])

        tril_sb = pp.tile([128, 128], BF16, name="tril_sb", tag="tril_sb")
        nc.sync.dma_start(tril_sb[:], io["tril"][:, :])

        # ---- persistent activations --------------------------------------
        qT = [pp.tile([128, S], BF16, name=f"qT{p}", tag=f"qT{p}") for p in range(2)]
        kT = [pp.tile([128, S], BF16, name=f"kT{p}", tag=f"kT{p}") for p in range(2)]
        oT = [pp.tile([128, S], BF16, name=f"oT{p}", tag=f"oT{p}") for p in range(2)]
        # V k-tiles: [128, 2, 193]; pair block b: cols 0:64 V_h(even), 64 ones,
        # 65 ones, 66:129 zeros, 129:193 V_h(odd).
        # Even-head lhsT [0:65] = [V|1] -> O at parts 0:64, denom at 64.
        # Odd-head lhsT [65:193] = [1|0*63|V] -> denom at part 0, O at 64:128.
        vt = [pp.tile([128, 2, 193], BF16, name=f"v{t}", tag=f"v{t}")
              for t in range(NKT)]

        # ---- QKV projections ---------------------------------------------
        for p in range(2):          # j pair-tile (2 heads each)
            for sc in range(NQ):    # s chunks of 512
                for (w_t, dst, bias, scale) in (
                    (wq_t, qT, bq_sb, 0.125),
                    (wk_t, kT, bk_sb, 1.0),
                ):
                    ps = proj_ps.tile([128, SC], F32, name="qk_ps", tag="proj")
                    for d in range(8):
                        nc.tensor.matmul(
                            ps[:],
                            lhsT=w_t[d][:, p * 128:(p + 1) * 128],
                            rhs=xT_t[d][:, sc * SC:(sc + 1) * SC],
                            start=(d == 0), stop=(d == 7),
                        )
                    nc.scalar.activation(
                        dst[p][:, sc * SC:(sc + 1) * SC], ps[:],
                        AF.Identity, bias=bias[:, p:p + 1], scale=scale,
                    )

        for st in range(NKT):
            ps = proj_ps.tile([128, JC], F32, name="v_ps", tag="proj")
            for d in range(8):
                nc.tensor.matmul(
                    ps[:],
                    lhsT=xT_t[d][:, st * 128:(st + 1) * 128],
                    rhs=wv_t[d][:],
                    start=(d == 0), stop=(d == 7),
                )
            ps3 = ps.rearrange("p (a c) -> p a c", a=2)      # a: pair, c: 2 heads
            bv3 = bv_bc.rearrange("p (a c) -> p a c", a=2)
            v3 = vt[st]
            # even heads -> cols 0:64 of each pair block
            nc.vector.tensor_tensor(
                out=v3[:, :, 0:64], in0=ps3[:, :, 0:64], in1=bv3[:, :, 0:64],
                op=ALU.add)
            # odd heads -> cols 129:193
            nc.vector.tensor_tensor(
                out=v3[:, :, 129:193], in0=ps3[:, :, 64:128], in1=bv3[:, :, 64:128],
                op=ALU.add)
            nc.vector.memset(v3[:, :, 64:66], 1.0)
            nc.vector.memset(v3[:, :, 66:129], 0.0)

        # ---- attention + output projection, q-chunk outer ----------------
        # Head pairs processed together: scores for even/odd head go to the
        # two halves of one [128, 1024] s-psum tile (adjacent 64-row matmuls
        # pack in the PE array), one strided exp covers both halves.
        for qi in range(NQ):
            for pr in range(2):
                nk = 4 * qi + 4

                def c0_of(ki):
                    return 128 * (ki - 4 * qi) if ki >= 4 * qi else 0

                o_e = o_ps.tile([128, SC], F32, name="o_e", tag="o")
                o_o = o_ps.tile([128, SC], F32, name="o_o", tag="o")
                # k-tiles in groups of 2: one [128, 2, 2, 512] scores tile and
                # a single exp per group (4 matmuls, 1 ACTIVATE).
                for ka in range(0, nk, 2):
                    kb = ka + 1
                    c0a, c0b = c0_of(ka), c0_of(kb)
                    sp = s_ps.tile([128, 2, 2, SC], F32, name="sp", tag="s")
                    for a, ki in ((0, ka), (1, kb)):
                        c0 = c0_of(ki)
                        for hf in range(2):
                            base = hf * 64
                            nc.tensor.matmul(
                                sp[:, a:a + 1, hf:hf + 1, c0:SC],
                                lhsT=kT[pr][base:base + 64, ki * 128:(ki + 1) * 128],
                                rhs=qT[pr][base:base + 64, qi * SC + c0:(qi + 1) * SC],
                                start=True, stop=True,
                            )
                    pt = p_pool.tile([128, 2, 2, SC], BF16, name="pt", tag="p")
                    # one exp over both k-tiles/heads at the wider col range;
                    # tile kb's [c0a:c0b] cols are unwritten psum -> zero after.
                    nc.scalar.activation(
                        pt[:, :, :, c0a:SC], sp[:, :, :, c0a:SC], AF.Exp)
                    if c0b > c0a:
                        nc.vector.memset(pt[:, 1:2, :, c0a:c0b], 0.0)
                    if kb >= 4 * qi:  # diagonal group: tril masks
                        for a, c0 in ((0, c0a), (1, c0b)):
                            for hf in range(2):
                                nc.vector.tensor_tensor(
                                    out=pt[:, a, hf, c0:c0 + 128],
                                    in0=pt[:, a, hf, c0:c0 + 128],
                                    in1=tril_sb[:], op=ALU.mult)
                    # even head lhsT [V|1] (65) -> O at parts 0:64, denom 64;
                    # odd head lhsT [1|0*63|V] (128) -> denom 0, O at 64:128.
                    for a, ki in ((0, ka), (1, kb)):
                        c0 = c0_of(ki)
                        nc.tensor.matmul(
                            o_e[0:65, c0:SC],
                            lhsT=vt[ki][:, pr, 0:65], rhs=pt[:, a, 0, c0:SC],
                            start=(ki == 0), stop=(ki == nk - 1),
                            skip_group_check=True,
                        )
                        nc.tensor.matmul(
                            o_o[0:128, c0:SC],
                            lhsT=vt[ki][:, pr, 65:193], rhs=pt[:, a, 1, c0:SC],
                            start=(ki == 0), stop=(ki == nk - 1),
                            skip_group_check=True,
                        )
                # normalization: copy PSUM out early (releases o banks), then
                # fast-recip the denom rows, broadcast, two multiplies with
                # matching base partitions (SBUF-SBUF ops require equal bases).
                u = small_pool.tile([128, 2 * SC], F32, name="u", tag="u")
                drow = small_pool.tile([1, SC], F32, name="drow", tag="drow")
                nc.vector.tensor_copy(u[0:64, 0:SC], o_e[0:64, :])
                # recip_approx_fast needs base-partition-0 input on HW
                nc.vector.tensor_copy(drow[0:1, :], o_e[64:65, :])
                nc.vector.tensor_copy(u[:, SC:2 * SC], o_o[:, :])
                rrow = small_pool.tile([1, 2 * SC], F32, name="rrow", tag="rrow")
                nc.vector.reciprocal_approx_fast(rrow[0:1, 0:SC], drow[0:1, :])
                nc.vector.reciprocal_approx_fast(rrow[0:1, SC:2 * SC], u[0:1, SC:2 * SC])
                rb = small_pool.tile([128, 2 * SC], F32, name="rb", tag="rb")
                nc.gpsimd.partition_broadcast(rb[:], rrow[:])
                nc.vector.tensor_tensor(
                    out=oT[pr][0:64, qi * SC:(qi + 1) * SC],
                    in0=u[0:64, 0:SC], in1=rb[0:64, 0:SC], op=ALU.mult)
                nc.vector.tensor_tensor(
                    out=oT[pr][64:128, qi * SC:(qi + 1) * SC],
                    in0=u[64:128, SC:2 * SC], in1=rb[64:128, SC:2 * SC],
                    op=ALU.mult)

            # output projection for the finished s-tiles
            for si in range(4 * qi, 4 * qi + 4):
                for mi in range(2):
                    yp = proj_ps.tile([128, SC], F32, name="yp", tag="proj")
                    for p in range(2):
                        nc.tensor.matmul(
                            yp[:],
                            lhsT=oT[p][:, si * 128:(si + 1) * 128],
                            rhs=wo_t[p][:, mi * SC:(mi + 1) * SC],
                            start=(p == 0), stop=(p == 1),
                        )
                    ys = y_pool.tile([128, SC], F32, name="ys", tag="y")
                    if mi == 0:
                        nc.scalar.copy(ys[:], yp[:])
                    else:
                        nc.vector.tensor_copy(ys[:], yp[:])
                    nc.sync.dma_start(
                        io["out"][si * 128:(si + 1) * 128, mi * SC:(mi + 1) * SC],
                        ys[:])


def build():
    nc = bacc.Bacc(
        "TRN2", target_bir_lowering=False, debug=False,
        enable_asserts=False, num_devices=N_CORES,
    )
    io = {
        "xT": nc.dram_tensor("xT", [D, S], BF16, kind="ExternalInput").ap(),
        "wq": nc.dram_tensor("wq", [D, JC], BF16, kind="ExternalInput").ap(),
        "wk": nc.dram_tensor("wk", [D, JC], BF16, kind="ExternalInput").ap(),
        "wv": nc.dram_tensor("wv", [D, JC], BF16, kind="ExternalInput").ap(),
        "wo": nc.dram_tensor("wo", [JC, D], BF16, kind="ExternalInput").ap(),
        "bqs": nc.dram_tensor("bqs", [128, 2], F32, kind="ExternalInput").ap(),
        "bks": nc.dram_tensor("bks", [128, 2], F32, kind="ExternalInput").ap(),
        "bvr": nc.dram_tensor("bvr", [1, JC], F32, kind="ExternalInput").ap(),
        "tril": nc.dram_tensor("tril", [128, 128], BF16, kind="ExternalInput").ap(),
        "out": nc.dram_tensor("out", [S, D], F32, kind="ExternalOutput").ap(),
    }
    with tile.TileContext(nc) as tc:
        _body(tc, io)
    nc.compile()
    return nc


def make_in_maps(x, Wq, bq, Wk, bk, Wv, bv, Wo):
    bf16 = ml_dtypes.bfloat16
    in_maps = []
    for c in range(N_CORES):
        b, g = divmod(c, HPC)
        j0 = JC * g
        xt = np.ascontiguousarray(np.asarray(x[b], np.float32).T).astype(bf16)
        in_maps.append({
            "xT": xt,
            "wq": np.asarray(Wq[:, j0:j0 + JC], np.float32).astype(bf16),
            "wk": np.asarray(Wk[:, j0:j0 + JC], np.float32).astype(bf16),
            "wv": np.asarray(Wv[:, j0:j0 + JC], np.float32).astype(bf16),
            "wo": np.asarray(Wo[j0:j0 + JC, :], np.float32).astype(bf16),
            "bqs": np.ascontiguousarray(
                (np.asarray(bq[j0:j0 + JC], np.float32) * 0.125).reshape(2, 128).T),
            "bks": np.ascontiguousarray(
                np.asarray(bk[j0:j0 + JC], np.float32).reshape(2, 128).T),
            "bvr": np.asarray(bv[j0:j0 + JC], np.float32).reshape(1, JC).copy(),
            # P^T tile is [k_part, q_free]: allowed iff q >= k -> upper triangular
            "tril": np.triu(np.ones((128, 128), np.float32)).astype(bf16),
        })
    return in_maps


_NC_CACHE = []


def run(x, Wq, bq, Wk, bk, Wv, bv, Wo, trace=False, **spmd_kwargs):
    if not _NC_CACHE:
        _NC_CACHE.append(build())
    nc = _NC_CACHE[0]
    in_maps = make_in_maps(x, Wq, bq, Wk, bk, Wv, bv, Wo)
    res = run_bass_kernel_spmd(
        nc, in_maps, core_ids=list(range(N_CORES)), trace=trace, **spmd_kwargs)
    out = np.zeros((B, S, D), np.float32)
    for c in range(N_CORES):
        b = c // HPC
        out[b] += res.results[c]["out"]
    return out, res


def kernel(x, Wq, bq, Wk, bk, Wv, bv, Wo):
    out, _ = run(x, Wq, bq, Wk, bk, Wv, bv, Wo, trace=False)
    return out


# revision 14
# speedup vs baseline: 1.2907x; 1.2907x over previous
"""Distributed causal multi-head attention for TRN2 (8 NeuronCores).

Problem: x[2,2048,1024] -> MHA(16 heads, dk=dv=64, causal) -> out[2,2048,1024].

Sharding: 2-way data parallel over batch x 4-way tensor parallel over heads.
Core c = 4*b + g handles batch b, heads 4g..4g+3 (columns 256g..256g+256 of
Wq/Wk/Wv, rows 256g..256g+256 of Wo). Each core computes a partial output
projection Y_bg = O_g @ Wo_g; the host sums the 4 partials per batch
(unsharding a sum-sharded tensor) and stacks the batches.

Device kernel (per core, transpose-free):
  - host passes x^T (d-major) in bf16, so QKV projections contract over d
    with no on-device transpose.
  - Q^T,K^T [j,s] layouts feed scores S^T = K^T.T @ Q^T directly; V in
    natural [k,v] layout feeds O^T = [V|1].T @ P^T; the appended ones
    column produces softmax denominators in the same matmul.
  - softmax without max-subtraction: scores are ~N(0,0.17) here, exp is
    safe in fp32; causal masking via k-tile skipping, column-narrowed
    matmuls and one 128x128 tril multiply per diagonal block.
  - out projection Y = O^T.T @ Wo lands in natural [s,m] layout for DMA.
"""

import numpy as np
import ml_dtypes

from concourse import bacc, mybir, tile
from concourse.bass_utils import run_bass_kernel_spmd

BF16 = mybir.dt.bfloat16
F32 = mybir.dt.float32
AF = mybir.ActivationFunctionType
ALU = mybir.AluOpType

B, S, D = 2, 2048, 1024
NH, DK = 16, 64
HPC = 4                      # heads per core
JC = HPC * DK                # 256 local q/k/v columns
N_CORES = 8
SC = 512                     # q-chunk (matmul moving free dim)
NQ = S // SC                 # 4 q-chunks
NKT = S // 128               # 16 k-tiles
NST = S // 128               # 16 s-tiles


def _body(tc, io):
    nc = tc.nc
    with (
        tc.tile_pool(name="persist", bufs=1) as pp,
        tc.tile_pool(name="proj_ps", bufs=2, space="PSUM") as proj_ps,
        tc.tile_pool(name="s_ps", bufs=2, space="PSUM") as s_ps,
        tc.tile_pool(name="o_ps", bufs=2, space="PSUM") as o_ps,
        tc.tile_pool(name="pbuf", bufs=4) as p_pool,
        tc.tile_pool(name="ybuf", bufs=4) as y_pool,
        tc.tile_pool(name="small", bufs=4) as small_pool,
    ):
        # ---- constant / weight loads -------------------------------------
        xT_t = []
        for d in range(8):
            t = pp.tile([128, S], BF16, name=f"xT{d}", tag=f"xT{d}")
            nc.sync.dma_start(t[:], io["xT"][d * 128:(d + 1) * 128, :])
            xT_t.append(t)

        def load_w(key, n_free):
            tiles = []
            for d in range(8):
                t = pp.tile([128, n_free], BF16, name=f"{key}{d}", tag=f"{key}{d}")
                nc.sync.dma_start(t[:], io[key][d * 128:(d + 1) * 128, :])
                tiles.append(t)
            return tiles

        wq_t = load_w("wq", JC)
        wk_t = load_w("wk", JC)
        wv_t = load_w("wv", JC)

        wo_t = []
        for p in range(2):
            t = pp.tile([128, D], BF16, name=f"wo{p}", tag=f"wo{p}")
            nc.sync.dma_start(t[:], io["wo"][p * 128:(p + 1) * 128, :])
            wo_t.append(t)

        bq_sb = pp.tile([128, 2], F32, name="bq_sb", tag="bq_sb")
        nc.sync.dma_start(bq_sb[:], io["bqs"][:, :])
        bk_sb = pp.tile([128, 2], F32, name="bk_sb", tag="bk_sb")
        nc.sync.dma_start(bk_sb[:], io["bks"][:, :])

        bv_row = pp.tile([1, JC], F32, name="bv_row", tag="bv_row")
        nc.sync.dma_start(bv_row[:], io["bvr"][:, :])
        bv_bc = pp.tile([128, JC], F32, name="bv_bc", tag="bv_bc")
        nc.gpsimd.partition_broadcast(bv_bc[:], bv_row[:])

        tril_sb = pp.tile([128, 128], BF16, name="tril_sb", tag="tril_sb")
        nc.sync.dma_start(tril_sb[:], io["tril"][:, :])

        # ---- persistent activations --------------------------------------
        qT = [pp.tile([128, S], BF16, name=f"qT{p}", tag=f"qT{p}") for p in range(2)]
        kT = [pp.tile([128, S], BF16, name=f"kT{p}", tag=f"kT{p}") for p in range(2)]
        oT = [pp.tile([128, S], BF16, name=f"oT{p}", tag=f"oT{p}") for p in range(2)]
        # V k-tiles: [128, 2, 193]; pair block b: cols 0:64 V_h(even), 64 ones,
        # 65 ones, 66:129 zeros, 129:193 V_h(odd).
        # Even-head lhsT [0:65] = [V|1] -> O at parts 0:64, denom at 64.
        # Odd-head lhsT [65:193] = [1|0*63|V] -> denom at part 0, O at 64:128.
        vt = [pp.tile([128, 2, 193], BF16, name=f"v{t}", tag=f"v{t}")
              for t in range(NKT)]

        # ---- QKV projections ---------------------------------------------
        for p in range(2):          # j pair-tile (2 heads each)
            for sc in range(NQ):    # s chunks of 512
                for (w_t, dst, bias, scale) in (
                    (wq_t, qT, bq_sb, 0.125),
                    (wk_t, kT, bk_sb, 1.0),
                ):
                    ps = proj_ps.tile([128, SC], F32, name="qk_ps", tag="proj")
                    for d in range(8):
                        nc.tensor.matmul(
                            ps[:],
                            lhsT=w_t[d][:, p * 128:(p + 1) * 128],
                            rhs=xT_t[d][:, sc * SC:(sc + 1) * SC],
                            start=(d == 0), stop=(d == 7),
                        )
                    nc.scalar.activation(
                        dst[p][:, sc * SC:(sc + 1) * SC], ps[:],
                        AF.Identity, bias=bias[:, p:p + 1], scale=scale,
                    )

        for st in range(NKT):
            ps = proj_ps.tile([128, JC], F32, name="v_ps", tag="proj")
            for d in range(8):
                nc.tensor.matmul(
                    ps[:],
                    lhsT=xT_t[d][:, st * 128:(st + 1) * 128],
                    rhs=wv_t[d][:],
                    start=(d == 0), stop=(d == 7),
                )
            ps3 = ps.rearrange("p (a c) -> p a c", a=2)      # a: pair, c: 2 heads
            bv3 = bv_bc.rearrange("p (a c) -> p a c", a=2)
            v3 = vt[st]
            # even heads -> cols 0:64 of each pair block
            nc.vector.tensor_tensor(
                out=v3[:, :, 0:64], in0=ps3[:, :, 0:64], in1=bv3[:, :, 0:64],
                op=ALU.add)
            # odd heads -> cols 129:193
            nc.vector.tensor_tensor(
                out=v3[:, :, 129:193], in0=ps3[:, :, 64:128], in1=bv3[:, :, 64:128],
                op=ALU.add)
            nc.vector.memset(v3[:, :, 64:66], 1.0)
            nc.vector.memset(v3[:, :, 66:129], 0.0)

        # ---- attention + output projection, q-chunk outer ----------------
        # Head pairs processed together: scores for even/odd head go to the
        # two halves of one [128, 1024] s-psum tile (adjacent 64-row matmuls
        # pack in the PE array), one strided exp covers both halves.
        for qi in range(NQ):
            for pr in range(2):
                nk = 4 * qi + 4

                def c0_of(ki):
                    return 128 * (ki - 4 * qi) if ki >= 4 * qi else 0

                o_e = o_ps.tile([128, SC], F32, name="o_e", tag="o")
                o_o = o_ps.tile([128, SC], F32, name="o_o", tag="o")
                for ki in range(nk):
                    c0 = c0_of(ki)
                    sp = s_ps.tile([128, 2, SC], F32, name="sp", tag="s")
                    for hf in range(2):
                        base = hf * 64
                        nc.tensor.matmul(
                            sp[:, hf:hf + 1, c0:SC],
                            lhsT=kT[pr][base:base + 64, ki * 128:(ki + 1) * 128],
                            rhs=qT[pr][base:base + 64, qi * SC + c0:(qi + 1) * SC],
                            start=True, stop=True,
                        )
                    pt = p_pool.tile([128, 2, SC], BF16, name="pt", tag="p")
                    nc.scalar.activation(
                        pt[:, :, c0:SC], sp[:, :, c0:SC], AF.Exp)
                    if ki >= 4 * qi:  # diagonal tile: tril masks
                        for hf in range(2):
                            nc.vector.tensor_tensor(
                                out=pt[:, hf, c0:c0 + 128],
                                in0=pt[:, hf, c0:c0 + 128],
                                in1=tril_sb[:], op=ALU.mult)
                    # even head lhsT [V|1] (65) -> O at parts 0:64, denom 64;
                    # odd head lhsT [1|0*63|V] (128) -> denom 0, O at 64:128.
                    nc.tensor.matmul(
                        o_e[0:65, c0:SC],
                        lhsT=vt[ki][:, pr, 0:65], rhs=pt[:, 0, c0:SC],
                        start=(ki == 0), stop=(ki == nk - 1),
                        skip_group_check=True,
                    )
                    nc.tensor.matmul(
                        o_o[0:128, c0:SC],
                        lhsT=vt[ki][:, pr, 65:193], rhs=pt[:, 1, c0:SC],
                        start=(ki == 0), stop=(ki == nk - 1),
                        skip_group_check=True,
                    )
                # normalization: copy PSUM out early (releases o banks), then
                # fast-recip the denom rows, broadcast, two multiplies with
                # matching base partitions (SBUF-SBUF ops require equal bases).
                u = small_pool.tile([128, 2 * SC], F32, name="u", tag="u")
                drow = small_pool.tile([1, SC], F32, name="drow", tag="drow")
                nc.vector.tensor_copy(u[0:64, 0:SC], o_e[0:64, :])
                # recip_approx_fast needs base-partition-0 input on HW
                nc.vector.tensor_copy(drow[0:1, :], o_e[64:65, :])
                nc.vector.tensor_copy(u[:, SC:2 * SC], o_o[:, :])
                rrow = small_pool.tile([1, 2 * SC], F32, name="rrow", tag="rrow")
                nc.vector.reciprocal_approx_fast(rrow[0:1, 0:SC], drow[0:1, :])
                nc.vector.reciprocal_approx_fast(rrow[0:1, SC:2 * SC], u[0:1, SC:2 * SC])
                rb = small_pool.tile([128, 2 * SC], F32, name="rb", tag="rb")
                nc.gpsimd.partition_broadcast(rb[:], rrow[:])
                nc.vector.tensor_tensor(
                    out=oT[pr][0:64, qi * SC:(qi + 1) * SC],
                    in0=u[0:64, 0:SC], in1=rb[0:64, 0:SC], op=ALU.mult)
                nc.vector.tensor_tensor(
                    out=oT[pr][64:128, qi * SC:(qi + 1) * SC],
                    in0=u[64:128, SC:2 * SC], in1=rb[64:128, SC:2 * SC],
                    op=ALU.mult)

            # output projection for the finished s-tiles
            for si in range(4 * qi, 4 * qi + 4):
                for mi in range(2):
                    yp = proj_ps.tile([128, SC], F32, name="yp", tag="proj")
                    for p in range(2):
                        nc.tensor.matmul(
                            yp[:],
                            lhsT=oT[p][:, si * 128:(si + 1) * 128],
                            rhs=wo_t[p][:, mi * SC:(mi + 1) * SC],
                            start=(p == 0), stop=(p == 1),
                        )
                    ys = y_pool.tile([128, SC], F32, name="ys", tag="y")
                    if mi == 0:
                        nc.scalar.copy(ys[:], yp[:])
                    else:
                        nc.vector.tensor_copy(ys[:], yp[:])
                    nc.sync.dma_start(
                        io["out"][si * 128:(si + 1) * 128, mi * SC:(mi + 1) * SC],
                        ys[:])


def build():
    nc = bacc.Bacc(
        "TRN2", target_bir_lowering=False, debug=False,
        enable_asserts=False, num_devices=N_CORES,
    )
    io = {
        "xT": nc.dram_tensor("xT", [D, S], BF16, kind="ExternalInput").ap(),
        "wq": nc.dram_tensor("wq", [D, JC], BF16, kind="ExternalInput").ap(),
        "wk": nc.dram_tensor("wk", [D, JC], BF16, kind="ExternalInput").ap(),
        "wv": nc.dram_tensor("wv", [D, JC], BF16, kind="ExternalInput").ap(),
        "wo": nc.dram_tensor("wo", [JC, D], BF16, kind="ExternalInput").ap(),
        "bqs": nc.dram_tensor("bqs", [128, 2], F32, kind="ExternalInput").ap(),
        "bks": nc.dram_tensor("bks", [128, 2], F32, kind="ExternalInput").ap(),
        "bvr": nc.dram_tensor("bvr", [1, JC], F32, kind="ExternalInput").ap(),
        "tril": nc.dram_tensor("tril", [128, 128], BF16, kind="ExternalInput").ap(),
        "out": nc.dram_tensor("out", [S, D], F32, kind="ExternalOutput").ap(),
    }
    with tile.TileContext(nc) as tc:
        _body(tc, io)
    nc.compile()
    return nc


def make_in_maps(x, Wq, bq, Wk, bk, Wv, bv, Wo):
    bf16 = ml_dtypes.bfloat16
    in_maps = []
    for c in range(N_CORES):
        b, g = divmod(c, HPC)
        j0 = JC * g
        xt = np.ascontiguousarray(np.asarray(x[b], np.float32).T).astype(bf16)
        in_maps.append({
            "xT": xt,
            "wq": np.asarray(Wq[:, j0:j0 + JC], np.float32).astype(bf16),
            "wk": np.asarray(Wk[:, j0:j0 + JC], np.float32).astype(bf16),
            "wv": np.asarray(Wv[:, j0:j0 + JC], np.float32).astype(bf16),
            "wo": np.asarray(Wo[j0:j0 + JC, :], np.float32).astype(bf16),
            "bqs": np.ascontiguousarray(
                (np.asarray(bq[j0:j0 + JC], np.float32) * 0.125).reshape(2, 128).T),
            "bks": np.ascontiguousarray(
                np.asarray(bk[j0:j0 + JC], np.float32).reshape(2, 128).T),
            "bvr": np.asarray(bv[j0:j0 + JC], np.float32).reshape(1, JC).copy(),
            # P^T tile is [k_part, q_free]: allowed iff q >= k -> upper triangular
            "tril": np.triu(np.ones((128, 128), np.float32)).astype(bf16),
        })
    return in_maps


_NC_CACHE = []


def run(x, Wq, bq, Wk, bk, Wv, bv, Wo, trace=False, **spmd_kwargs):
    if not _NC_CACHE:
        _NC_CACHE.append(build())
    nc = _NC_CACHE[0]
    in_maps = make_in_maps(x, Wq, bq, Wk, bk, Wv, bv, Wo)
    res = run_bass_kernel_spmd(
        nc, in_maps, core_ids=list(range(N_CORES)), trace=trace, **spmd_kwargs)
    out = np.zeros((B, S, D), np.float32)
    for c in range(N_CORES):
        b = c // HPC
        out[b] += res.results[c]["out"]
    return out, res


def kernel(x, Wq, bq, Wk, bk, Wv, bv, Wo):
    out, _ = run(x, Wq, bq, Wk, bk, Wv, bv, Wo, trace=False)
    return out
